# revision 12
# baseline (speedup 1.0000x reference)
"""Trainium2 Bass kernel for nn_CrossAttentionModule (cross-attention token
re-weighting): per batch, L2-normalize 196 tokens of class/query feats over
C=640 channels, corr = ct @ qt^T, tiny MLP on corr means -> kernel vector,
softmax(corr @ kernel / T) -> per-token attention, out = feat * (1 + attn).

Key structure: corr is only consumed through rank-1 contractions, so the
kernel computes the raw gram G = cls^T @ [cls|qry] once per batch (fp32r,
full PE rate) and folds all L2-normalization factors into the small
per-token vectors instead of scaling the gram.  Small ops are batched
across groups of 4 batches via identity-matmul transposes.

Sharding: pure data parallel, B=512 -> 64 batches on each of 8 NeuronCores.
"""
import numpy as np

try:
    import concourse.bass as bass
except ImportError:  # fresh grading dir: toolchain lives in /opt/trn_rl_repo
    import sys
    sys.path.insert(0, "/opt/trn_rl_repo")
    import concourse.bass as bass

import bass_rust
import concourse.mybir as mybir
from concourse import tile
from concourse.bass_utils import run_bass_kernel_spmd
from concourse.vector_clock import ScopedClock

F32 = mybir.dt.float32
F32R = mybir.dt.float32r
I32 = mybir.dt.int32
AF = mybir.ActivationFunctionType
ALU = mybir.AluOpType

C = 640          # channels
T = 196          # tokens (14*14)
NCH = 5          # C / 128 chunks
TA, TB = 128, 68  # token chunks
INV_TEMP = 40.0  # 1 / 0.025
N_CORES = 8
GRP = 4          # batches per group (small-op amortization)


def _patched_drain_and_barrier(self, tick_clock, wait_clock):
    # Walrus here rejects >2 sync waits on one instruction ("Too many sync
    # wait commands"). Emit one wait_ge per semaphore, then a bare drain.
    probe = self.nc.sync.nop()
    wait_clock.add_sem_waits(probe.ins, ScopedClock({None: tick_clock.global_clock}))
    si = probe.ins.sync_info
    waits = list(si.on_wait) if si is not None else []
    probe.ins.sync_info = bass_rust.SyncInfo(
        on_wait=[], on_update=list(si.on_update) if si is not None else []
    )
    handles = {h.name: h for h in self.sems.allocated().values()}
    for w in waits:
        self.nc.sync.wait_ge(handles[w.ant_name], w.wait_value)
    self.nc.sync.drain()
    self.nc.all_engine_barrier()
    popped = self.nc._tile_sem_poison_stack.pop()
    assert popped is self._sem_poison
    self.nc.clear_and_free_semaphores(list(self.sems.allocated().values()))
    self.nc.all_engine_barrier()


tile.TileContext._drain_and_barrier = _patched_drain_and_barrier

WAIT_LIMIT = 1  # max sem waits walrus accepts on one instruction


def _split_waits(nc, limit=None):
    if limit is None:
        limit = WAIT_LIMIT
    """Walrus rejects instructions carrying more than `limit` sync waits.
    Move excess waits onto same-engine NoOps inserted just before."""
    n_split = 0
    for fn in nc.m.functions:
        for blk in fn.blocks:
            il = blk.instructions  # live list
            idx = 0
            while idx < len(il):
                inst = il[idx]
                si = getattr(inst, "sync_info", None)
                if si is not None and len(si.on_wait) > limit:
                    waits = list(si.on_wait)
                    inst.sync_info = bass_rust.SyncInfo(
                        on_wait=waits[:limit], on_update=list(si.on_update))
                    extra = waits[limit:]
                    pos = idx
                    for j in range(0, len(extra), limit):
                        nop = mybir.InstNoOp(
                            name=f"wsplit-{nc.next_id()}", ins=[], outs=[])
                        nop.engine = inst.engine
                        nop.sync_info = bass_rust.SyncInfo(
                            on_wait=extra[j:j + limit], on_update=[])
                        il.insert(pos, nop)
                        pos += 1
                        idx += 1
                        n_split += 1
                idx += 1
    return n_split


USE_BITCAST = True   # feed DMA-loaded fp32 to PE as fp32r via bitcast
USE_BCAST_AP = True  # 0-stride AP to broadcast bc over the 5 channel chunks


def _bcast_n(ap, n):
    """View a [128, 2, 196] AP as [128, 2, n, 196] with 0-stride over n."""
    lst = ap.ap  # [[stride, size], ...]; first entry is the partition dim
    new = [lst[0], lst[1], [0, n], lst[2]]
    return bass.AP(tensor=ap.tensor, offset=ap.offset, ap=new)


def build_nc(bpc, trace_sim=False, reps=1, loop_reps=1, wait_limit=1):
    nc = bass.Bass("TRN2", target_bir_lowering=False, debug=False,
                   num_devices=N_CORES)
    cf = nc.dram_tensor("cf", [bpc, C, T], F32, kind="ExternalInput").ap()
    qf = nc.dram_tensor("qf", [bpc, C, T], F32, kind="ExternalInput").ap()
    # stacked MLP weights: col/row block 0 = query branch, 1 = class branch
    w1s = nc.dram_tensor("w1s", [T, 64], F32, kind="ExternalInput").ap()
    w2s = nc.dram_tensor("w2s", [64, T], F32, kind="ExternalInput").ap()
    b1s = nc.dram_tensor("b1s", [64, 1], F32, kind="ExternalInput").ap()
    # b2cols[:, q]: q = (qb2-a, cb2-a, qb2-b, cb2-b), chunk a = tokens 0:128
    b2c = nc.dram_tensor("b2c", [128, 4], F32, kind="ExternalInput").ap()
    onec = nc.dram_tensor("onec", [128, 1], F32, kind="ExternalInput").ap()
    oner = nc.dram_tensor("oner", [1, 128], F32, kind="ExternalInput").ap()
    i128 = nc.dram_tensor("i128", [128, 128], F32, kind="ExternalInput").ap()
    oner392 = nc.dram_tensor("oner392", [1, 392], F32, kind="ExternalInput").ap()
    co = nc.dram_tensor("co", [bpc, C, T], F32, kind="ExternalOutput").ap()
    qo = nc.dram_tensor("qo", [bpc, C, T], F32, kind="ExternalOutput").ap()

    with tile.TileContext(nc, trace_sim=trace_sim) as tc:
        with (
            tc.tile_pool(name="const", bufs=1) as cp,
            tc.tile_pool(name="xp", bufs=2) as xp,
            tc.tile_pool(name="sqp", bufs=2) as sqp,
            tc.tile_pool(name="xrp", bufs=3) as xrp,
            tc.tile_pool(name="op", bufs=4) as op,
            tc.tile_pool(name="gsb", bufs=6) as gsb,
            tc.tile_pool(name="smp", bufs=3) as smp,
            tc.tile_pool(name="grp_sb", bufs=2) as grp_sb,
            tc.tile_pool(name="normps", bufs=2, space="PSUM") as normps,
            tc.tile_pool(name="rowps", bufs=3, space="PSUM") as rowps,
            tc.tile_pool(name="gps", bufs=2, space="PSUM") as gps,
            tc.tile_pool(name="qpps", bufs=1, space="PSUM") as qpps,
            tc.tile_pool(name="bcps", bufs=1, space="PSUM") as bcps,
        ):
            # ---- persistent constants ----
            w1asb = cp.tile([TA, 64], F32)
            w1bsb = cp.tile([TB, 64], F32)
            w2sb = cp.tile([64, T], F32)
            b1_sb = cp.tile([64, 1], F32)
            b2_sb = cp.tile([128, 4], F32)
            onec_sb = cp.tile([128, 1], F32)
            oner_sb = cp.tile([1, 128], F32)
            id_sb = cp.tile([128, 128], F32)
            oner392_sb = cp.tile([1, 392], F32)
            nc.sync.dma_start(w1asb[:, :], w1s[0:TA, :])
            nc.sync.dma_start(w1bsb[:, :], w1s[TA:T, :])
            nc.sync.dma_start(w2sb[:, :], w2s[:, :])
            nc.sync.dma_start(b1_sb[:, :], b1s[:, :])
            nc.sync.dma_start(b2_sb[:, :], b2c[:, :])
            nc.sync.dma_start(onec_sb[:, :], onec[:, :])
            nc.sync.dma_start(oner_sb[:, :], oner[:, :])
            nc.sync.dma_start(id_sb[:, :], i128[:, :])
            nc.sync.dma_start(oner392_sb[:, :], oner392[:, :])
            # fp32r copies (fp32r matmul operands must come from an engine op)
            onec_r = cp.tile([128, 1], F32R)
            oner_r = cp.tile([1, 128], F32R)
            id_r = cp.tile([128, 128], F32R)
            ones392_r = cp.tile([1, 392], F32R)
            nc.vector.tensor_copy(onec_r[:, :], onec_sb[:, :])
            nc.vector.tensor_copy(oner_r[:, :], oner_sb[:, :])
            nc.vector.tensor_copy(id_r[:, :], id_sb[:, :])
            nc.vector.tensor_copy(ones392_r[:, :], oner392_sb[:, :])

            # pn layout: channel c -> (partition p, chunk n), c = p*NCH + n.
            # Channel reductions are permutation-invariant and the store uses
            # the inverse mapping, so every DMA is one contiguous block/batch.
            cfr = cf.rearrange("b (p n) t -> p b (n t)", p=128)
            qfr = qf.rearrange("b (p n) t -> p b (n t)", p=128)
            cor = co.rearrange("b (p n) t -> p b (n t)", p=128)
            qor = qo.rearrange("b (p n) t -> p b (n t)", p=128)

            n_groups = (bpc + GRP - 1) // GRP

            def load_group(g):
                b0 = g * GRP
                nb = min(GRP, bpc - b0)
                x4 = xp.tile([128, GRP, 2, NCH * T], F32, name="x4")
                nc.sync.dma_start(x4[:, 0:nb, 0, :], cfr[:, b0:b0 + nb, :])
                nc.sync.dma_start(x4[:, 0:nb, 1, :], qfr[:, b0:b0 + nb, :])
                return x4

            def emit_group(g, x4):
                b0 = g * GRP
                nb = min(GRP, bpc - b0)
                jj = range(nb)

                # Shared per-group PSUM bank (bank-level dependency tracking
                # serializes accesses in emission order == dataflow order):
                #   cols 0:16    ssq columns  [128, 4(q), 4(b)]
                #   cols 16:32   m columns    [128, 4(q), 4(b)]
                #   cols 32:48   logit cols   [128, 4(c), 4(b)]
                #   cols 440:472 per-batch MLP chain [64.., 8] each
                # q-order: (cls-a, qry-a, cls-b, qry-b)-flavored
                grpps_t = normps.tile([128, 512], F32, space="PSUM",
                                      tag="grp", name="grpps")
                sc = grpps_t[:, 0:16].rearrange("p (b q) -> p b q", b=4)
                mcg = grpps_t[:, 16:32].rearrange("p (b q) -> p b q", b=4)
                lg = grpps_t[:, 32:48].rearrange("p (b c) -> p b c", b=4)

                QSL = ((slice(0, TA), slice(0, TA)),          # q0 cls-a
                       (slice(0, TA), slice(T, T + TA)),      # q1 qry-a
                       (slice(0, TB), slice(TA, T)),          # q2 cls-b
                       (slice(0, TB), slice(T + TA, 2 * T)))  # q3 qry-b

                # --- norms: sq -> per-batch ssq row -> columns -> rsqrt ---
                one1 = onec_sb[0:1, 0:1]
                for j in jj:
                    sq = sqp.tile([128, 2, NCH * T], F32R, name="sq")
                    nc.gpsimd.tensor_mul(sq[:, :, :], x4[:, j], x4[:, j])
                    srow = rowps.tile([1, 2 * T], F32, tag="row", name="srow")
                    srv = srow[:, :].rearrange("o (i t) -> o i t", i=2)
                    for n in range(NCH):
                        nc.tensor.matmul(srv, onec_r[:, :],
                                         sq[:, :, n * T:(n + 1) * T],
                                         start=(n == 0), stop=(n == NCH - 1))
                    srow_sb = smp.tile([1, 2 * T], F32, tag="ssq", bufs=4,
                                       name="srow_sb")
                    nc.vector.tensor_copy(srow_sb[:, :], srow[:, :])
                    for q, (psl, fsl) in enumerate(QSL):
                        nc.tensor.matmul(sc[psl, j, q:q + 1],
                                         srow_sb[:, fsl], one1,
                                         start=True, stop=True)

                # quake rsqrt + 2 Newton steps on [128, 4, 4]
                sh = smp.tile([128, GRP, 4], I32, tag="nw", name="sh")
                nc.vector.tensor_scalar(sh[:, :, :], sc.bitcast(I32),
                                        1, None, ALU.logical_shift_right)
                y0i = smp.tile([128, GRP, 4], I32, tag="nw2", name="y0i")
                nc.vector.tensor_scalar(y0i[:, :, :], sh[:, :, :],
                                        -1, 0x5F3759DF, ALU.mult, ALU.add)
                y = y0i[:, :, :].bitcast(F32)
                for it in range(2):
                    last = it == 1
                    a2 = smp.tile([128, GRP, 4], F32, tag="nwa", name="a2")
                    nc.vector.tensor_mul(a2[:, :, :], y, y)
                    bsy = smp.tile([128, GRP, 4], F32, tag="nwb", name="bsy")
                    nc.vector.tensor_mul(bsy[:, :, :], a2[:, :, :], sc)
                    cny = smp.tile([128, GRP, 4], F32, tag="nwc", name="cny")
                    nc.vector.tensor_scalar(cny[:, :, :], bsy[:, :, :],
                                            -0.5, 1.5, ALU.mult, ALU.add)
                    yn = smp.tile([128, GRP, 4], F32,
                                  tag="nwy" if last else "nwy0", name="yn")
                    nc.vector.tensor_mul(yn[:, :, :], y, cny[:, :, :])
                    y = yn[:, :, :]
                rcq = yn  # [128, 4(b), 4(q)] F32; q = (ca, qa, cb, qb)

                # --- per-batch gram + m columns ---
                g_sbs = []
                for j in jj:
                    rc = rcq[:, j, :]  # [128, 4] contiguous; (ca, qa, cb, qb)

                    # fp32r rounding producer for the gram operands (walrus
                    # rejects DMA-fed fp32r); ACT is the least-loaded engine
                    xr = xrp.tile([128, 2, NCH * T], F32R, name="xr")
                    nc.scalar.copy(xr[:, :, :], x4[:, j])

                    # raw gram G[t, (i, u)] = sum_c cls[c, t] * x[c, i, u]
                    ga = gps.tile([TA, 2, T], F32, space="PSUM", tag="g",
                                  name="ga")
                    gb = gps.tile([TB, 2, T], F32, space="PSUM", tag="g",
                                  name="gb")
                    for out_ps, tsl in ((ga, slice(0, TA)), (gb, slice(TA, T))):
                        for n in range(NCH):
                            nc.tensor.matmul(
                                out_ps[:, :, :],
                                xr[:, 0, n * T + tsl.start:n * T + tsl.stop],
                                xr[:, :, n * T:(n + 1) * T],
                                start=(n == 0), stop=(n == NCH - 1))
                    g_a = gsb.tile([TA, T], F32R, name="g_a")
                    g_b = gsb.tile([TB, T], F32R, name="g_b")
                    nc.vector.tensor_copy(g_a[:, :], ga[:, 1, :])
                    nc.vector.tensor_copy(g_b[:, :], gb[:, 1, :])

                    # G^T via PE transpose: qp[(u-chunk), t]
                    qp = qpps.tile([128, 2 * T], F32R, space="PSUM", tag="qp",
                                   name="qp")
                    nc.tensor.transpose(qp[0:TA, 0:TA], g_a[:, 0:TA],
                                        id_r[:, :])
                    nc.tensor.transpose(qp[0:TA, TA:T], g_b[:, 0:TA],
                                        id_r[0:TB, 0:TB])
                    nc.tensor.transpose(qp[0:TB, T:T + TA], g_a[:, TA:T],
                                        id_r[:, :])
                    nc.tensor.transpose(qp[0:TB, T + TA:2 * T], g_b[:, TA:T],
                                        id_r[0:TB, 0:TB])
                    qp_a = gsb.tile([TA, T], F32R, name="qp_a")
                    qp_b = gsb.tile([TB, T], F32R, name="qp_b")
                    nc.vector.tensor_copy(qp_a[:, :], qp[0:TA, 0:T])
                    nc.vector.tensor_copy(qp_b[:, :], qp[0:TB, T:2 * T])
                    g_sbs.append((g_a, g_b, qp_a, qp_b))

                    # m columns, q-order (m_qry-a, m_cls-a, m_qry-b, m_cls-b)
                    # m_qry[t] = sum_u G^T[u,t] rn_q[u] (1/T folded into w1)
                    # m_cls[u] = sum_t G[t,u] rn_c[t]
                    for q, (A_a, A_b, ba, bb, psl) in enumerate((
                            (qp_a[:, 0:TA], qp_b[:, 0:TA],
                             rc[:, 1:2], rc[0:TB, 3:4], slice(0, TA)),
                            (g_a[:, 0:TA], g_b[:, 0:TA],
                             rc[:, 0:1], rc[0:TB, 2:3], slice(0, TA)),
                            (qp_a[:, TA:T], qp_b[:, TA:T],
                             rc[:, 1:2], rc[0:TB, 3:4], slice(0, TB)),
                            (g_a[:, TA:T], g_b[:, TA:T],
                             rc[:, 0:1], rc[0:TB, 2:3], slice(0, TB)))):
                        nc.tensor.matmul(mcg[psl, j, q:q + 1],
                                         A_a.bitcast(F32), ba,
                                         start=True, stop=False)
                        nc.tensor.matmul(mcg[psl, j, q:q + 1],
                                         A_b.bitcast(F32), bb,
                                         start=False, stop=True)

                # scale m by the other branch's rn == rcq q-order
                mcol = grp_sb.tile([128, GRP, 4], F32, tag="mcol",
                                   name="mcol")
                nc.vector.tensor_mul(mcol[:, :, :], mcg, rcq[:, :, :])

                # --- per-batch MLP + logit columns ---
                for j in jj:
                    rc = rcq[:, j, :]
                    g_a, g_b, qp_a, qp_b = g_sbs[j]
                    # per-batch chain region: cols 0:2 MLP1, 2:6 MLP2
                    ch = grpps_t[:, 440 + 8 * j:448 + 8 * j]
                    nc.tensor.matmul(ch[0:64, 0:2], w1asb[:, :],
                                     mcol[:, j, 0:2],
                                     start=True, stop=False)
                    nc.tensor.matmul(ch[0:64, 0:2], w1bsb[0:TB, :],
                                     mcol[0:TB, j, 2:4],
                                     start=False, stop=True)
                    z = smp.tile([64, 2], F32, tag="z", name="z")
                    nc.scalar.activation(z[:, :], ch[0:64, 0:2], AF.Relu,
                                         bias=b1_sb[:, :], scale=1.0)

                    # MLP layer 2 (w2 pre-scaled by INV_TEMP), per branch
                    # k columns 2:6: (k_qry-a, k_cls-a, k_qry-b, k_cls-b)
                    nc.tensor.matmul(ch[0:TA, 2:3], w2sb[0:32, 0:TA],
                                     z[0:32, 0:1], start=True, stop=True)
                    nc.tensor.matmul(ch[0:TA, 3:4], w2sb[32:64, 0:TA],
                                     z[32:64, 1:2], start=True, stop=True)
                    nc.tensor.matmul(ch[0:TB, 4:5], w2sb[0:32, TA:T],
                                     z[0:32, 0:1], start=True, stop=True)
                    nc.tensor.matmul(ch[0:TB, 5:6], w2sb[32:64, TA:T],
                                     z[32:64, 1:2], start=True, stop=True)
                    # wk = (k + b2) * rn_other; b2 pre-scaled by INV_TEMP
                    wk0 = smp.tile([128, 4], F32, tag="wk0", name="wk0")
                    nc.vector.tensor_add(wk0[:, :], ch[:, 2:6], b2_sb[:, :])
                    wk = smp.tile([128, 4], F32, tag="wk", name="wk")
                    nc.vector.tensor_mul(wk[:, :], wk0[:, :], rc)

                    # logit columns, c-order (lc-a, lq-a, lc-b, lq-b)
                    # lc[t] = sum_u G^T[u,t] wk_c[u]; lq[u] = sum_t G wk_q[t]
                    for c, (A_a, A_b, ba, bb, psl) in enumerate((
                            (qp_a[:, 0:TA], qp_b[:, 0:TA],
                             wk[:, 1:2], wk[0:TB, 3:4], slice(0, TA)),
                            (g_a[:, 0:TA], g_b[:, 0:TA],
                             wk[:, 0:1], wk[0:TB, 2:3], slice(0, TA)),
                            (qp_a[:, TA:T], qp_b[:, TA:T],
                             wk[:, 1:2], wk[0:TB, 3:4], slice(0, TB)),
                            (g_a[:, TA:T], g_b[:, TA:T],
                             wk[:, 0:1], wk[0:TB, 2:3], slice(0, TB)))):
                        nc.tensor.matmul(lg[psl, j, c:c + 1],
                                         A_a.bitcast(F32), ba,
                                         start=True, stop=False)
                        nc.tensor.matmul(lg[psl, j, c:c + 1],
                                         A_b.bitcast(F32), bb,
                                         start=False, stop=True)

                # scale logits by own-branch rn (c-order == rcq q-order)
                lgs = grp_sb.tile([128, GRP, 4], F32R, tag="lgs", name="lgs")
                nc.vector.tensor_mul(lgs[:, :, :], lg, rcq[:, :, :])

                # --- per-batch: logit cols -> row, softmax, bc, output ---
                for j in jj:
                    b = b0 + j
                    lraw = rowps.tile([1, 2 * T], F32, tag="row", name="lraw")
                    for cq, (psl, fsl) in enumerate(QSL):
                        src = (0, 2, 1, 3)[cq]
                        nc.tensor.matmul(lraw[:, fsl], lgs[psl, j, src:src + 1],
                                         id_r[psl, psl],
                                         start=True, stop=True)

                    # softmax without max-subtraction (|logits| < 1)
                    es = smp.tile([1, 2, T], F32R, tag="es", bufs=4,
                                  name="es")
                    ssum = smp.tile([1, 2], F32, tag="ssum", bufs=4,
                                    name="ssum")
                    lrv = lraw[:, :].rearrange("o (i t) -> o i t", i=2)
                    for br in range(2):
                        nc.scalar.activation(es[:, br, :], lrv[:, br, :],
                                             AF.Exp, scale=1.0,
                                             accum_out=ssum[:, br:br + 1])
                    rs = smp.tile([1, 2], F32, tag="rs", bufs=4, name="rs")
                    nc.vector.reciprocal(rs[:, :], ssum[:, :])
                    for br in range(2):
                        nc.vector.tensor_scalar_mul(es[:, br, :],
                                                    es[:, br, :],
                                                    rs[:, br:br + 1])

                    # bc = 1 + attn (PE rank-1 broadcast); out = x * bc
                    bc = rowps.tile([128, 2, T], F32, space="PSUM", tag="row",
                                    name="bc")
                    nc.tensor.matmul(bc[:, :, :], oner_r[:, :],
                                     ones392_r[:, :].rearrange(
                                         "o (i t) -> o i t", i=2),
                                     start=True, stop=False,
                                     skip_group_check=True)
                    nc.tensor.matmul(bc[:, :, :], oner_r[:, :],
                                     es[:, :, :],
                                     start=False, stop=True,
                                     skip_group_check=True)
                    bc_sb = smp.tile([128, 2, T], F32, tag="bcsb", bufs=2,
                                     name="bc_sb")
                    nc.scalar.copy(bc_sb[:, :, :], bc[:, :, :])
                    o = op.tile([128, 2, NCH, T], F32, name="o")
                    xv = x4[:, j].rearrange("p i (n t) -> p i n t", n=NCH)
                    if USE_BCAST_AP:
                        nc.vector.tensor_mul(o[:, :, :, :], xv,
                                             _bcast_n(bc_sb[:, :, :], NCH))
                    else:
                        for n in range(NCH):
                            nc.vector.tensor_mul(o[:, :, n, :],
                                                 x4[:, j, :, n * T:(n + 1) * T],
                                                 bc_sb[:, :, :])
                    nc.scalar.dma_start(cor[:, b, :],
                                        o[:, 0].rearrange("p n t -> p (n t)"))
                    nc.scalar.dma_start(qor[:, b, :],
                                        o[:, 1].rearrange("p n t -> p (n t)"))

            def emit_all():
                AHEAD = 1
                tiles = {}

                def ensure(g):
                    if g not in tiles and g < n_groups:
                        tiles[g] = load_group(g)

                for g0 in range(AHEAD + 1):
                    ensure(g0)
                for g in range(n_groups):
                    ensure(g + AHEAD + 1)
                    emit_group(g, tiles[g])
                    del tiles[g]

            if loop_reps > 1:
                with tc.For_i(0, loop_reps, 1):
                    emit_all()
            else:
                emit_all()
    _split_waits(nc, wait_limit)
    return nc


def _consts():
    return {
        "onec": np.ones((128, 1), np.float32),
        "oner": np.ones((1, 128), np.float32),
        "i128": np.eye(128, dtype=np.float32),
        "oner392": np.ones((1, 392), np.float32),
    }


_CACHE = {}


def prep_in_maps(class_feat, query_feat, cw1, cb1, cw2, cb2, qw1, qb1, qw2, qb2):
    B = class_feat.shape[0]
    bpc = B // N_CORES
    cfull = np.ascontiguousarray(np.asarray(class_feat, np.float32).reshape(B, C, T))
    qfull = np.ascontiguousarray(np.asarray(query_feat, np.float32).reshape(B, C, T))
    w1s = (np.concatenate([np.asarray(qw1), np.asarray(cw1)], axis=1)
           / T).astype(np.float32)
    w2s = (np.concatenate([np.asarray(qw2), np.asarray(cw2)], axis=0)
           * INV_TEMP).astype(np.float32)
    b1s = np.concatenate([np.asarray(qb1), np.asarray(cb1)])[:, None].astype(np.float32)
    b2c = np.zeros((128, 4), np.float32)
    b2c[:, 0] = np.asarray(qb2)[0:TA] * INV_TEMP
    b2c[:, 1] = np.asarray(cb2)[0:TA] * INV_TEMP
    b2c[0:TB, 2] = np.asarray(qb2)[TA:T] * INV_TEMP
    b2c[0:TB, 3] = np.asarray(cb2)[TA:T] * INV_TEMP
    consts = _consts()
    in_maps = []
    for c in range(N_CORES):
        sl = slice(c * bpc, (c + 1) * bpc)
        in_maps.append({
            "cf": cfull[sl], "qf": qfull[sl],
            "w1s": w1s, "w2s": w2s, "b1s": b1s, "b2c": b2c, **consts,
        })
    return in_maps


def kernel(class_feat, query_feat, cw1, cb1, cw2, cb2, qw1, qb1, qw2, qb2):
    B = class_feat.shape[0]
    bpc = B // N_CORES
    if bpc not in _CACHE:
        _CACHE[bpc] = build_nc(bpc)
    nc = _CACHE[bpc]
    in_maps = prep_in_maps(class_feat, query_feat, cw1, cb1, cw2, cb2,
                           qw1, qb1, qw2, qb2)
    res = run_bass_kernel_spmd(nc, in_maps, core_ids=list(range(N_CORES)))
    S = int(np.sqrt(T))
    co = np.concatenate([res.results[c]["co"] for c in range(N_CORES)], axis=0)
    qo = np.concatenate([res.results[c]["qo"] for c in range(N_CORES)], axis=0)
    return (co.reshape(B, C, S, S), qo.reshape(B, C, S, S))
